# revision 9
# baseline (speedup 1.0000x reference)
"""DepthCueExtractor TRN2 kernel (bf16 I/O, communication-free).

out[b,u,y,x,n] = mean_v(lfi[b,u,y,x,v]) * s_mask[b,n] * h_mask[b,n,y]
  s_mask[b,n]   = sum_{h,w} f_maps[b,h,w,n]
  h_mask[b,n,y] = colsum[b,y,n] / max_w colsum[b,w,n]
  colsum[b,w,n] = sum_h f_maps[b,h,w,n]

Sharding: 8 cores = (batch b in 0..3) x (H-half in 0..1), data-parallel on the
output. Each core reads its lfi slice plus its own 128-column W-half of
f_maps[b] in bf16. The global per-(b,n) stats (sum over all w, max over all
w of colsum) also need the OTHER W-half: instead of a cross-core exchange
(the cost model charges a flat 15us per collective, which would gate every
output store), each core redundantly loads the pair's half in fp8-e4m3
(+2.1MB, +5.8us of DMA). The fp8 data only feeds sum/max reductions where
quantization noise averages out; the half that scales the output (h_mask
numerator) stays bf16.

All bulk HBM traffic is bf16 (host down/up-casts around the device call):
the problem's rel-err tolerance (2e-2) is ~2.5x above the ~8e-3 worst-case
quantization error, and it halves the DMA-bound runtime vs fp32. colsum is
reduced entirely on the PE (accumulating ones-matmuls, contraction rows are
free in the cost model). The output phase multiplies mlf[y,x] (broadcast
over the OUTER n dim, innermost x contiguous) against a materialized n-major
expansion wfx[y,n,x] of the per-(y,n) weight, keeping every operand 2-byte +
innermost-contiguous so the DVE runs in its 2x mode; the expansion is built
by log2(W) doubling copies in the 4x copy mode. The HBM output is n-major
([U, HY, N, W], host transposes back during unshard) so n-quarter store
tiles keep a 512B-contiguous innermost HBM run at full DMA rate.
~49.3MB HBM traffic per core at 360GB/s aggregate -> ~139us/core."""

import numpy as np
from ml_dtypes import bfloat16

import concourse.bass as bass
import concourse.bacc as bacc
import concourse.bass_isa as bass_isa
import concourse.mybir as mybir
import concourse.tile as tile
from concourse.bass_utils import run_bass_kernel_spmd

F32 = mybir.dt.float32
BF16 = mybir.dt.bfloat16
FP8 = mybir.dt.float8e4
FP8_NP = mybir.dt.np(FP8)

B, U, H, W, V, N = 4, 9, 256, 256, 9, 64
HY = H // 2


def build_kernel_body(nc, tc, lfi_s, fm, fm8, out_s):
    with (
        tc.tile_pool(name="const", bufs=1) as const_pool,
        tc.tile_pool(name="fmp", bufs=2) as fm_pool,
        tc.tile_pool(name="psum", bufs=1, space="PSUM") as psum_pool,
        tc.tile_pool(name="stats", bufs=1) as stats_pool,
        tc.tile_pool(name="lfip", bufs=1) as lfi_pool,
        tc.tile_pool(name="mlfp", bufs=1) as mlf_pool,
        tc.tile_pool(name="wfxp", bufs=1) as wfx_pool,
        tc.tile_pool(name="outp", bufs=2) as out_pool,
    ):
        ones = const_pool.tile([128, 1], BF16)
        nc.vector.memset(ones[:], 1.0)
        ones8 = const_pool.tile([128, 1], FP8)
        nc.vector.memset(ones8[:], 1.0)

        # ---- Phase A: colsum[w, n] = sum_h fm[h, w, n], own half in bf16,
        # pair's half in fp8. Reduced on the PE alone: per (w-chunk, n) two
        # accumulating matmuls contract the two 128-row h-halves.
        WQ = 64  # w-chunk width (PE out base partition must be 0/32/64)
        cs_psum = psum_pool.tile([128, N], F32)
        cs8_psum = psum_pool.tile([128, N], F32)

        def colsum(dram, dt, one, psum_t, tag):
            for wq in range(128 // WQ):
                sl = slice(wq * WQ, (wq + 1) * WQ)
                f0 = fm_pool.tile([128, WQ, N], dt, name=f"f0{tag}_{wq}",
                                  tag=f"f0{tag}", bufs=2)
                f1 = fm_pool.tile([128, WQ, N], dt, name=f"f1{tag}_{wq}",
                                  tag=f"f1{tag}", bufs=2)
                nc.sync.dma_start(out=f0[:], in_=dram[0:128, sl, :])
                nc.sync.dma_start(out=f1[:], in_=dram[128:256, sl, :])
                for n in range(N):
                    nc.tensor.matmul(
                        out=psum_t[sl, n : n + 1],
                        lhsT=f0[:, :, n],
                        rhs=one[:, 0:1],
                        start=True,
                        stop=False,
                    )
                    nc.tensor.matmul(
                        out=psum_t[sl, n : n + 1],
                        lhsT=f1[:, :, n],
                        rhs=one[:, 0:1],
                        start=False,
                        stop=True,
                    )

        colsum(fm, BF16, ones, cs_psum, "b")
        colsum(fm8, FP8, ones8, cs8_psum, "q")

        hp = tc.high_priority
        with hp():
            cs_sb = stats_pool.tile([128, N], F32)
            nc.vector.tensor_copy(out=cs_sb[:], in_=cs_psum[:])
            cs8_sb = stats_pool.tile([128, N], F32)
            nc.vector.tensor_copy(out=cs8_sb[:], in_=cs8_psum[:])

        # ---- Phase A2: global stats, fully core-local. Partition p holds
        # colsum for one w of each half; elementwise add/max then a
        # partition reduction covers all 256 w's.
        with hp():
            comb_s = stats_pool.tile([128, N], F32)
            nc.vector.tensor_add(out=comb_s[:], in0=cs_sb[:], in1=cs8_sb[:])
            comb_m = stats_pool.tile([128, N], F32)
            nc.vector.tensor_max(out=comb_m[:], in0=cs_sb[:], in1=cs8_sb[:])
            red_s = stats_pool.tile([128, N], F32)
            nc.gpsimd.partition_all_reduce(
                red_s[:], comb_s[:], 128, bass_isa.ReduceOp.add
            )
            red_m = stats_pool.tile([128, N], F32)
            nc.gpsimd.partition_all_reduce(
                red_m[:], comb_m[:], 128, bass_isa.ReduceOp.max
            )

            # wf[y, n] = colsum[y, n] * s_mask[n] / (V * max_w colsum[w, n])
            m9 = stats_pool.tile([128, N], F32)
            nc.vector.tensor_scalar_mul(m9[:], red_m[:], float(V))
            rec = stats_pool.tile([128, N], F32)
            nc.vector.reciprocal(out=rec[:], in_=m9[:])
            sn = stats_pool.tile([128, N], F32)
            nc.vector.tensor_mul(out=sn[:], in0=red_s[:], in1=rec[:])
            wf = stats_pool.tile([128, N], F32)
            nc.vector.tensor_mul(out=wf[:], in0=cs_sb[:], in1=sn[:])
            wf_bf = stats_pool.tile([128, N], BF16)
            nc.vector.tensor_copy(out=wf_bf[:], in_=wf[:])

        # ---- Phase B: issue all lfi loads up front (after fm loads in DMA
        # order). V-mean reduces run on the DVE, interleaved with the output
        # multiplies below.
        lts = []
        for u in range(U):
            lt = lfi_pool.tile([128, W, V], BF16, name=f"lt{u}", tag=f"lt{u}")
            nc.sync.dma_start(out=lt[:], in_=lfi_s[u])
            lts.append(lt)

        mlf = [
            mlf_pool.tile([128, W], BF16, name=f"mlf{u}", tag=f"mlf{u}")
            for u in range(U)
        ]
        red_pool = mlf_pool  # v-sum scratch lives beside mlf

        def reduce_u(u):
            # V-sum as an add-tree on the otherwise-idle GPSIMD engine,
            # keeping the DVE free for the output multiplies (gpsimd has no
            # free-axis tensor_reduce). Intermediates stay f32 (same gpsimd
            # cost — it charges elements, not bytes) so mlf rounds to bf16
            # only once.
            lt = lts[u]
            t1 = red_pool.tile([128, W, 4], F32, name=f"t1_{u}", tag="t1",
                               bufs=2)
            t2 = red_pool.tile([128, W, 2], F32, name=f"t2_{u}", tag="t2",
                               bufs=2)
            t3 = red_pool.tile([128, W], F32, name=f"t3_{u}", tag="t3",
                               bufs=2)
            nc.gpsimd.tensor_add(out=t1[:], in0=lt[:, :, 0:4], in1=lt[:, :, 4:8])
            nc.gpsimd.tensor_add(out=t2[:], in0=t1[:, :, 0:2], in1=t1[:, :, 2:4])
            nc.gpsimd.tensor_add(out=t3[:], in0=t2[:, :, 0], in1=lt[:, :, 8])
            nc.gpsimd.tensor_add(out=mlf[u][:], in0=t3[:], in1=t2[:, :, 1])

        # ---- Phase B2: expand wf to wfx[y, n, x] = wf[y, n] (n-major, x
        # contiguous) by log-doubling copies (4x DVE copy mode for w >= 2).
        wfx = wfx_pool.tile([128, N, W], BF16)
        seed_dst = bass.AP(
            tensor=wfx.tensor, offset=wfx.offset, ap=[wfx.ap[0], [W, N]]
        )
        nc.vector.tensor_copy(out=seed_dst, in_=wf_bf[:])
        w = 1
        while w < W:
            nc.vector.tensor_copy(
                out=wfx[:, :, w : 2 * w], in_=wfx[:, :, 0:w]
            )
            w *= 2

        # ---- Phase C: out_s[u, y, n, x] = mlf[u][y, x] * wfx[y, n, x], with
        # the V-mean reduces interleaved between output multiplies. The HBM
        # output is n-major ([U, HY, N, W]); the host transposes back during
        # unshard. n-quarter tiles keep every store's innermost HBM run at
        # 512B (full DMA rate) and pipeline at ~1MB granularity.
        NQ = 16
        def emit_tile(u, n0):
            ot = out_pool.tile([128, NQ, W], BF16, name=f"ot{u}_{n0}",
                               tag="ot", bufs=3)
            msl = mlf[u][:]
            m_b = bass.AP(
                tensor=msl.tensor, offset=msl.offset,
                ap=[msl.ap[0], [0, NQ], msl.ap[1]],
            )
            nc.vector.tensor_mul(out=ot[:], in0=m_b, in1=wfx[:, n0 : n0 + NQ, :])
            nc.sync.dma_start(out=out_s[u, :, n0 : n0 + NQ, :], in_=ot[:])

        reduce_u(0)
        for u in range(U):
            for nh in range(N // NQ):
                emit_tile(u, nh * NQ)
            if u + 1 < U:
                reduce_u(u + 1)


def build_nc():
    nc = bacc.Bacc("TRN2", target_bir_lowering=False, debug=True)
    lfi_s = nc.dram_tensor("lfi_s", [U, HY, W, V], BF16, kind="ExternalInput")
    fm = nc.dram_tensor("fm", [H, HY, N], BF16, kind="ExternalInput")
    fm8 = nc.dram_tensor("fm8", [H, HY, N], FP8, kind="ExternalInput")
    out_s = nc.dram_tensor("out_s", [U, HY, N, W], BF16, kind="ExternalOutput")
    with tile.TileContext(nc) as tc:
        build_kernel_body(nc, tc, lfi_s, fm, fm8, out_s)
    nc.compile()
    return nc


_CACHE = {}


def make_in_maps(lfi, f_maps):
    lfi16 = lfi.astype(bfloat16)
    fm16 = f_maps.astype(bfloat16)
    fm8 = f_maps.astype(FP8_NP)
    in_maps = []
    for c in range(8):
        b, half = divmod(c, 2)
        other = 1 - half
        lf = np.ascontiguousarray(lfi16[b, :, half * HY : (half + 1) * HY])
        fmc = np.ascontiguousarray(fm16[b][:, half * HY : (half + 1) * HY, :])
        fm8c = np.ascontiguousarray(fm8[b][:, other * HY : (other + 1) * HY, :])
        in_maps.append({"lfi_s": lf, "fm": fmc, "fm8": fm8c})
    return in_maps


def kernel(lfi, f_maps):
    lfi = np.asarray(lfi, dtype=np.float32)
    f_maps = np.asarray(f_maps, dtype=np.float32)
    if "nc" not in _CACHE:
        _CACHE["nc"] = build_nc()
    nc = _CACHE["nc"]
    res = run_bass_kernel_spmd(nc, make_in_maps(lfi, f_maps), list(range(8)))
    out = np.empty((B, U, H, W, N), np.float32)
    for c in range(8):
        b, half = divmod(c, 2)
        # device output is [U, HY, N, W]; unshard transposes back to
        # [U, HY, W, N]
        out[b, :, half * HY : (half + 1) * HY] = (
            res.results[c]["out_s"].astype(np.float32).transpose(0, 1, 3, 2)
        )
    return out
